# revision 1
# baseline (speedup 1.0000x reference)
"""Data-parallel AE+CNF forward on 8 Trainium2 NeuronCores.

Shards the batch dim of x across the 8 cores (256 rows each); all
parameters (convs, linears, hypernet) are tiny and replicated. The
final scalar mean(logp_x) is an equal-shard mean-of-means reduction.
Returns the full-batch (recon, x_probs) like the reference.
"""
import numpy as np
import jax
import jax.numpy as jnp
from jax import lax
from functools import partial

B = 2048; IMG = 28; K = 3; CIN = 1; HC = 32; LC = 16; WID = 64; HID = 64
LIS = IMG - 3 * (K - 1)           # 22
FLAT = LIS * LIS * HC             # 15488
BLK = WID * LC                    # 1024
T1 = 10.0; T0 = 0.0; NSTEP = 20
VAR = 0.1
NC = 8                            # cores
BS = B // NC                      # 256 per core


def _conv(x, w, b):
    y = lax.conv_general_dilated(x, w, (1, 1), 'VALID',
                                 dimension_numbers=('NCHW', 'OIHW', 'NCHW'))
    return y + b[None, :, None, None]


def _convT(x, w, b):
    wf = jnp.flip(w, (2, 3)).transpose(1, 0, 2, 3)
    y = lax.conv_general_dilated(x, wf, (1, 1), [(K - 1, K - 1), (K - 1, K - 1)],
                                 dimension_numbers=('NCHW', 'OIHW', 'NCHW'))
    return y + b[None, :, None, None]


def _shard_forward(x, c0w, c0b, c1w, c1b, c2w, c2b, elw, elb, dlw, dlb,
                   t0w, t0b, t1w, t1b, t2w, t2b, h1w, h1b, h2w, h2b, h3w, h3b):
    h = jnp.tanh(_conv(x, c0w, c0b))
    h = jnp.tanh(_conv(h, c1w, c1b))
    h = jnp.tanh(_conv(h, c2w, c2b))
    z1 = h.reshape(h.shape[0], -1) @ elw + elb
    d = jnp.tanh(z1 @ dlw + dlb)
    d = d.reshape(-1, HC, LIS, LIS)
    d = jnp.tanh(_convT(d, t0w, t0b))
    d = jnp.tanh(_convT(d, t1w, t1b))
    recon = _convT(d, t2w, t2b)

    def hyper(t):
        p = jnp.tanh(jnp.reshape(jnp.asarray(t, jnp.float32), (1, 1)) @ h1w + h1b)
        p = jnp.tanh(p @ h2w + h2b)
        p = (p @ h3w + h3b).reshape(-1)
        W = p[:BLK].reshape(WID, LC)
        U = p[BLK:2 * BLK].reshape(WID, LC) * jax.nn.sigmoid(p[2 * BLK:3 * BLK].reshape(WID, LC))
        Bb = p[3 * BLK:]
        return W, Bb, U

    def f(t, z):
        W, Bb, U = hyper(t)
        hh = jnp.tanh(jnp.einsum('bd,wd->wb', z, W) + Bb[:, None])
        dz = jnp.einsum('wb,wd->bd', hh, U) / WID
        s = jnp.sum(W * U, axis=1)
        tr = jnp.mean((1.0 - hh * hh) * s[:, None], axis=0)
        return dz, -tr[:, None]

    dt = (T0 - T1) / NSTEP

    def step(state, i):
        z, lp = state
        t = T1 + i * dt
        k1z, k1l = f(t, z)
        k2z, k2l = f(t + 0.5 * dt, z + 0.5 * dt * k1z)
        k3z, k3l = f(t + 0.5 * dt, z + 0.5 * dt * k2z)
        k4z, k4l = f(t + dt, z + dt * k3z)
        z = z + dt / 6.0 * (k1z + 2 * k2z + 2 * k3z + k4z)
        lp = lp + dt / 6.0 * (k1l + 2 * k2l + 2 * k3l + k4l)
        return (z, lp), None

    lp1 = jnp.zeros((z1.shape[0], 1), z1.dtype)
    (z0, lp0), _ = lax.scan(step, (z1, lp1), jnp.arange(NSTEP, dtype=jnp.float32))

    logp = -0.5 * (LC * jnp.log(2 * jnp.pi) + LC * jnp.log(VAR) + jnp.sum(z0 * z0, -1) / VAR)
    shard_mean = jnp.mean(logp - lp0[:, 0])
    return recon, shard_mean


_PARAM_NAMES = ('c0w', 'c0b', 'c1w', 'c1b', 'c2w', 'c2b', 'elw', 'elb',
                'dlw', 'dlb', 't0w', 't0b', 't1w', 't1b', 't2w', 't2b',
                'h1w', 'h1b', 'h2w', 'h2b', 'h3w', 'h3b')

_pmapped = None


def _get_pmapped():
    global _pmapped
    if _pmapped is None:
        _pmapped = jax.pmap(_shard_forward,
                            in_axes=(0,) + (None,) * len(_PARAM_NAMES),
                            devices=jax.devices()[:NC])
    return _pmapped


def kernel(**inputs):
    x = np.ascontiguousarray(inputs['x'], dtype=np.float32)
    params = [np.asarray(inputs[n], dtype=np.float32) for n in _PARAM_NAMES]
    xs = x.reshape(NC, BS, CIN, IMG, IMG)
    fn = _get_pmapped()
    recon_sh, shard_means = fn(xs, *params)
    recon = np.asarray(recon_sh).reshape(B, CIN, IMG, IMG).astype(np.float32)
    x_probs = np.float32(np.asarray(shard_means, dtype=np.float64).mean())
    return recon, jnp.asarray(x_probs)
